# revision 1
# baseline (speedup 1.0000x reference)
"""Trainium2 Bass kernel: Llama-style attention block (prefill, start_pos=0).

Reference computation (per problem):
  q = x @ wq; k = x @ wk; v = x @ wv          (DIM=4096 -> 32 q-heads / 8 kv-heads, hd=128)
  rope(q, k) with interleaved (even, odd) pairs using freqs_cos/freqs_sin inputs
  scores = q @ k^T / sqrt(128) + mask ; p = softmax(scores) ; o = p @ v (GQA 4x)
  out = o @ wo

Distribution: tensor-parallel over heads on 8 cores. Core c owns q-heads
4c..4c+3 and kv-head c (GQA groups align with the core boundary), i.e.
wq/wk/wv are sharded column-wise and wo row-wise. Each core computes a
full-shape partial of the output projection; the host sums the 8 partials
(the row-parallel all-reduce, done on the host at unshard time).

Layout strategy on-chip (per core):
  - host passes x transposed (xT [4096, 2048]) so Q^T/K^T/V^T come out of the
    PE in dim-major layout [dims, seq], which is exactly the operand layout
    attention needs (contraction over head_dim = partition axis).
  - RoPE: wq/wk columns are permuted on the host so each head's rotation
    pairs (even, odd) become (first 64, last 64) rows. The pair swap
    [a;b] -> [-b;a] is then a constant 128x128 matmul on the PE, and the
    cos/sin combine is 3 elementwise DVE ops. Dot products are invariant
    under the permutation so scores match the reference exactly.
  - scores are computed transposed (S^T [k, q] blocks): softmax denominators
    become ones-vector matmuls on the PE (partition-axis reduction), exp
    runs on the scalar engine reading PSUM directly, and P^T feeds the
    P@V matmul with no transposes anywhere. V is transposed to seq-major
    once (16 PE transposes).
  - all matmul operands use float32r (E8M11): full PE rate at N>=256 with
    ~2.4e-4 element precision; PSUM accumulation stays fp32.
"""

import math

import numpy as np

import concourse.bass as bass
import concourse.mybir as mybir
import concourse.tile as tile
from concourse import bacc, bass_utils

DIM = 4096
N_HEADS = 32
N_KV = 8
HD = 128
SEQ = 2048
NCORES = 8
HPC = N_HEADS // NCORES          # q heads per core
QD = HPC * HD                    # 512 q-dims per core
SCALE = 1.0 / math.sqrt(HD)
NEG = -1.0e30

NQC = SEQ // 512                 # q chunks of 512
NKB = SEQ // 128                 # k blocks of 128
NKC = DIM // 128                 # contraction chunks of 128
XG = 2                           # kc chunks per x DMA group

F32 = mybir.dt.float32
F32R = mybir.dt.float32r
EXP = mybir.ActivationFunctionType.Exp

_PROG_CACHE = {}


def _build_program(mask_mode: str):
    """mask_mode: 'causal' (skip upper blocks, add triangular diagonal
    masks), 'none' (no masking), 'full' (add arbitrary maskT blocks)."""
    assert mask_mode in ("causal", "none", "full")
    nc = bacc.Bacc("TRN2", target_bir_lowering=False, debug=False,
                   num_devices=NCORES)

    xT = nc.dram_tensor("xT", [DIM, SEQ], F32R, kind="ExternalInput").ap()
    wq = nc.dram_tensor("wq", [DIM, QD], F32R, kind="ExternalInput").ap()
    wk = nc.dram_tensor("wk", [DIM, HD], F32R, kind="ExternalInput").ap()
    wv = nc.dram_tensor("wv", [DIM, HD], F32R, kind="ExternalInput").ap()
    wo = nc.dram_tensor("wo", [QD, DIM], F32R, kind="ExternalInput").ap()
    cos2 = nc.dram_tensor("cos2", [HD, SEQ], F32, kind="ExternalInput").ap()
    sin2 = nc.dram_tensor("sin2", [HD, SEQ], F32, kind="ExternalInput").ap()
    rmat = nc.dram_tensor("rmat", [HD, HD], F32R, kind="ExternalInput").ap()
    ident = nc.dram_tensor("ident", [128, 128], F32R, kind="ExternalInput").ap()
    ones_col_d = nc.dram_tensor("ones_col", [128, 1], F32R,
                                kind="ExternalInput").ap()
    ones_row_d = nc.dram_tensor("ones_row", [1, 128], F32R,
                                kind="ExternalInput").ap()
    if mask_mode == "causal":
        dmask_d = nc.dram_tensor("dmask", [4, 128, 512], F32,
                                 kind="ExternalInput").ap()
    if mask_mode == "full":
        maskT_d = nc.dram_tensor("maskT", [SEQ, SEQ], F32,
                                 kind="ExternalInput").ap()
    out = nc.dram_tensor("out", [SEQ, DIM], F32, kind="ExternalOutput").ap()

    with tile.TileContext(nc) as tc:
        with tc.tile_pool(name="persist", bufs=1) as pp:
            # ---- persistent tiles ----
            qt = [pp.tile([128, SEQ], F32R, name=f"qt{h}") for h in range(HPC)]
            kt = pp.tile([128, SEQ], F32R)
            vs = pp.tile([128, SEQ], F32R)        # seq-major V, block i at cols i*128
            rmat_sb = pp.tile([128, 128], F32R)
            ident_sb = pp.tile([128, 128], F32R)
            nc.sync.dma_start(ident_sb[:], ident[:])
            nc.sync.dma_start(rmat_sb[:], rmat[:])
            ones_sb = pp.tile([128, 1], F32R)
            nc.sync.dma_start(ones_sb[:], ones_col_d[:])
            onesrow = pp.tile([1, 128], F32R)
            nc.sync.dma_start(onesrow[:], ones_row_d[:])
            if mask_mode == "causal":
                dmask_sb = pp.tile([128, 4, 512], F32)
                nc.gpsimd.dma_start(dmask_sb[:],
                                    dmask_d.rearrange("r p q -> p r q"))

            # ================= Phase 1: QKV projections + RoPE =================
            psp = tc.alloc_tile_pool(name="ps", bufs=1, space="PSUM")
            # dummy matmuls on the identity tile keep the PE activity monitor
            # warm through the initial DMA window (else the first ~3.5us of
            # real matmuls run at half clock)
            warm = psp.tile([128, 128], F32, tag="vtr", bufs=1)
            for _ in range(50):
                nc.tensor.matmul(warm[:], ident_sb[:], ident_sb[:],
                                 start=True, stop=True)
            with tc.tile_pool(name="w1", bufs=1) as wp:
                # chunked weight loads so the first matmuls start after ~2MB
                wq_sb = wp.tile([128, NKC, QD], F32R)
                wk_sb = wp.tile([128, NKC, HD], F32R)
                wv_sb = wp.tile([128, NKC, HD], F32R)
                for k0, k1 in ((0, 2), (2, 8), (8, 20), (20, 32)):
                    ksl = slice(k0, k1)
                    for wsb, wdr in ((wq_sb, wq), (wk_sb, wk), (wv_sb, wv)):
                        w3 = wdr.rearrange("(kc p) m -> p kc m", p=128)
                        nc.sync.dma_start(wsb[:, ksl, :], w3[:, ksl, :])
                cos_sb = wp.tile([128, SEQ], F32)
                sin_sb = wp.tile([128, SEQ], F32)

                xT3 = xT.rearrange("(kc p) s -> kc p s", p=128)

                def drain_copy(m, pt, n):
                    # pass A: free the accumulator bank and launch the PE part
                    # (swap matmul / transposes) with nothing else in between,
                    # so neither the DVE nor the PE stream head-of-line blocks
                    # on rope arithmetic of an earlier head
                    raw = wp.tile([128, 512], F32R, tag="raw", bufs=6,
                                  name=f"raw{n}_{m}")
                    if m % 2 == 0:
                        nc.scalar.copy(raw[:], pt[:])
                    else:
                        nc.vector.tensor_copy(raw[:], pt[:])
                    if m <= HPC:
                        swp = psp.tile([128, 512], F32,
                                       tag=("aux" if m % 2 == 0 else "vtr"),
                                       bufs=1, name=f"swp{n}_{m}")
                        nc.tensor.matmul(swp[:], rmat_sb[:], raw[:],
                                         start=True, stop=True)
                        return raw, swp
                    pvts = []
                    for b in range(4):
                        pvt = psp.tile([128, 128], F32R, tag="vtr", bufs=1,
                                       name=f"pvt{n}_{b}")
                        nc.tensor.transpose(pvt[:], raw[:, b * 128:(b + 1) * 128],
                                            ident_sb[:])
                        pvts.append(pvt)
                    return raw, pvts

                def drain_rope_a(m, raw, pe_out, n, nsl):
                    # pass B1: drain the swap/transpose PSUM banks straight
                    # into the destination (partial rope: dst = swp*sin)
                    if m <= HPC:
                        dst = qt[m] if m < HPC else kt
                        nc.vector.tensor_mul(dst[:, nsl], pe_out[:],
                                             sin_sb[:, nsl])
                        return None
                    for b, pvt in enumerate(pe_out):
                        i = n * 4 + b
                        nc.vector.tensor_copy(vs[:, i * 128:(i + 1) * 128],
                                              pvt[:])
                    return None

                def drain_rope_b(m, raw, tmp, n, nsl):
                    # pass B2: finish the rope combine (dst += raw*cos)
                    if m <= HPC:
                        dst = qt[m] if m < HPC else kt
                        tmp2 = wp.tile([128, 512], F32, tag="ropetmp", bufs=2,
                                       name=f"tmp{n}_{m}")
                        nc.vector.tensor_mul(tmp2[:], raw[:], cos_sb[:, nsl])
                        nc.vector.tensor_add(dst[:, nsl], dst[:, nsl], tmp2[:])

                for n in range(NQC):
                    nsl = slice(n * 512, (n + 1) * 512)
                    waves = [list(range(HPC + 2))]
                    for w_i, mlist in enumerate(waves):
                        pts = {m: psp.tile([128, 512], F32, tag="big", bufs=6,
                                           name=f"pt{n}_{w_i}_{m}")
                               for m in mlist}
                        for g in range(NKC // XG):
                            xg = wp.tile([128, XG, 512], F32R, tag="xg", bufs=5)
                            nc.scalar.dma_start(
                                xg[:], xT3[g * XG:(g + 1) * XG, :, nsl]
                                .rearrange("kc p s -> p kc s"))
                            for kk in range(XG):
                                kc = g * XG + kk
                                st, sp = (kc == 0), (kc == NKC - 1)
                                for m in mlist:
                                    if m < HPC:
                                        w_ap = wq_sb[:, kc, m * 128:(m + 1) * 128]
                                    elif m == HPC:
                                        w_ap = wk_sb[:, kc, :]
                                    else:
                                        w_ap = wv_sb[:, kc, :]
                                    nc.tensor.matmul(pts[m][:], w_ap,
                                                     xg[:, kk, :],
                                                     start=st, stop=sp)
                        if n == 0 and w_i == 0:
                            # cos/sin are first needed here; deferring their
                            # 2MB load keeps early HBM bandwidth for x/weights
                            nc.sync.dma_start(cos_sb[:], cos2[:])
                            nc.sync.dma_start(sin_sb[:], sin2[:])
                        handles = {m: drain_copy(m, pts[m], n) for m in mlist}
                        tmps = {m: drain_rope_a(m, *handles[m], n, nsl)
                                for m in mlist}
                        for m in mlist:
                            drain_rope_b(m, handles[m][0], tmps[m], n, nsl)

            # ========== Phase 2+3: attention interleaved with out-proj ==========
            # One merged stream: after attention finishes q-chunk j, the
            # output projection for seq blocks 4j..4j+3 runs while chunk j+1's
            # attention pipeline fills — keeps the PE dense (HAM stays warm).
            with tc.tile_pool(name="pp2", bufs=1) as pp2:
                attn = [pp2.tile([128, SEQ], F32R, name=f"attn{h}")
                        for h in range(HPC)]

                with tc.tile_pool(name="att", bufs=1) as ap_:
                    wo_sb = ap_.tile([128, HPC, DIM], F32R)
                    wo3 = wo.rearrange("(kc p) n -> p kc n", p=128)
                    for g in range(2):
                        nc.sync.dma_start(wo_sb[:, :, g * 2048:(g + 1) * 2048],
                                          wo3[:, :, g * 2048:(g + 1) * 2048])

                    def wo_block(m):
                        # one 128-row seq block of the output projection
                        msl = slice(m * 128, (m + 1) * 128)
                        for w4 in range(4):
                            yps = [psp.tile([128, 512], F32, tag="big", bufs=6,
                                            name=f"yp{m}_{w4}_{i}")
                                   for i in range(2)]
                            for kc in range(HPC):
                                for i in range(2):
                                    ncol = w4 * 2 + i
                                    nc.tensor.matmul(
                                        yps[i][:], attn[kc][:, msl],
                                        wo_sb[:, kc, ncol * 512:(ncol + 1) * 512],
                                        start=(kc == 0), stop=(kc == HPC - 1))
                            for i in range(2):
                                ncol = w4 * 2 + i
                                ysb = ap_.tile([128, 512], F32, tag="ysb",
                                               bufs=6)
                                nc.vector.tensor_copy(ysb[:], yps[i][:])
                                nc.sync.dma_start(
                                    out[msl, ncol * 512:(ncol + 1) * 512],
                                    ysb[:])

                    for j in range(NQC):
                        jsl = slice(j * 512, (j + 1) * 512)
                        nblk = 4 * j + 4 if mask_mode == "causal" else NKB
                        for h in range(HPC):
                            # previous chunk's out-proj traced ahead of this
                            # h-chain: dense PE filler under the softmax chain
                            if j > 0:
                                wo_block(4 * (j - 1) + h)
                            dn = psp.tile([1, 512], F32, tag="aux", bufs=1,
                                          name=f"dn{h}_{j}")
                            pv = psp.tile([128, 512], F32, tag="big", bufs=6,
                                          name=f"pv{h}_{j}")
                            for i in range(nblk):
                                r = i - 4 * j
                                off = 128 * r if (mask_mode == "causal" and r > 0) else 0
                                qof = j * 512 + off
                                stp = psp.tile([128, 512], F32, tag="big", bufs=6,
                                               name=f"st{h}_{j}_{i}")
                                nc.tensor.matmul(stp[:, off:],
                                                 kt[:, i * 128:(i + 1) * 128],
                                                 qt[h][:, qof:(j + 1) * 512],
                                                 start=True, stop=True)
                                if mask_mode == "causal" and r >= 0:
                                    nc.vector.tensor_add(
                                        stp[:, off:], stp[:, off:],
                                        dmask_sb[:, r, off:])
                                elif mask_mode == "full":
                                    mt = ap_.tile([128, 512], F32, tag="mt", bufs=3)
                                    nc.sync.dma_start(
                                        mt[:], maskT_d[i * 128:(i + 1) * 128, jsl])
                                    nc.vector.tensor_add(stp[:], stp[:], mt[:])
                                pexp = ap_.tile([128, 512], F32R, tag="pexp",
                                                bufs=6, name=f"pexp{h}_{j}_{i}")
                                nc.scalar.activation(pexp[:, off:], stp[:, off:],
                                                     EXP, scale=SCALE)
                                nc.tensor.matmul(dn[:, off:], ones_sb[:],
                                                 pexp[:, off:],
                                                 start=(i == 0),
                                                 stop=(i == nblk - 1))
                                nc.tensor.matmul(pv[:, off:],
                                                 vs[:, i * 128:(i + 1) * 128],
                                                 pexp[:, off:],
                                                 start=(i == 0),
                                                 stop=(i == nblk - 1))
                            rcp = ap_.tile([1, 512], F32, tag="rcp", bufs=2)
                            nc.vector.reciprocal_approx_fast(rcp[:], dn[:])
                            rcpr = ap_.tile([1, 512], F32R, tag="rcpr", bufs=2)
                            nc.vector.tensor_copy(rcpr[:], rcp[:])
                            bc = psp.tile([128, 512], F32, tag="vtr", bufs=1,
                                          name=f"bc{h}_{j}")
                            nc.tensor.matmul(bc[:], onesrow[:], rcpr[:],
                                             start=True, stop=True)
                            bcs = ap_.tile([128, 512], F32, tag="bcs", bufs=2)
                            nc.vector.tensor_copy(bcs[:], bc[:])
                            nc.vector.tensor_mul(attn[h][:, jsl], pv[:], bcs[:])

                    for m in range(4 * (NQC - 1), 4 * NQC):
                        wo_block(m)
            psp.release()

    nc.compile()
    return nc


def get_program(mask_mode: str):
    if mask_mode not in _PROG_CACHE:
        _PROG_CACHE[mask_mode] = _build_program(mask_mode)
    return _PROG_CACHE[mask_mode]


# ====================== host-side preparation ======================

_PERM128 = np.concatenate([np.arange(0, 128, 2), np.arange(1, 128, 2)])


def _perm_cols(w: np.ndarray, n_heads: int) -> np.ndarray:
    """Permute each head's 128 columns: even dims first, odd dims last."""
    cols = np.concatenate([h * 128 + _PERM128 for h in range(n_heads)])
    return w[:, cols]


def _classify_mask(mask: np.ndarray) -> str:
    if not np.any(mask):
        return "none"
    iu = np.triu_indices(SEQ, 1)
    upper = mask[iu]
    lower_ok = not np.any(np.tril(mask))
    upper_ok = bool(np.all(np.isneginf(upper) | (upper <= -1e9)))
    if lower_ok and upper_ok:
        return "causal"
    return "full"


def _host_inputs(x, wq, wk, wv, wo, freqs_cos, freqs_sin, mask):
    x2 = np.ascontiguousarray(x.reshape(SEQ, DIM).T)        # xT [DIM, SEQ]
    wq_p = _perm_cols(np.asarray(wq, np.float32), N_HEADS)
    wk_p = _perm_cols(np.asarray(wk, np.float32), N_KV)
    wv_ = np.asarray(wv, np.float32)
    wo_ = np.asarray(wo, np.float32)

    cosT = np.asarray(freqs_cos, np.float32).T              # [64, SEQ]
    sinT = np.asarray(freqs_sin, np.float32).T
    cos2 = np.ascontiguousarray(np.concatenate([cosT, cosT], 0))  # [128, SEQ]
    sin2 = np.ascontiguousarray(np.concatenate([sinT, sinT], 0))

    rmat = np.zeros((HD, HD), np.float32)
    rmat[np.arange(64) + 64, np.arange(64)] = -1.0   # swp[:64] = -raw[64:]
    rmat[np.arange(64), np.arange(64) + 64] = 1.0    # swp[64:] = raw[:64]
    ident = np.eye(128, dtype=np.float32)

    mask = np.asarray(mask, np.float32)
    mode = _classify_mask(mask)

    common = {"xT": x2, "cos2": cos2, "sin2": sin2, "rmat": rmat,
              "ident": ident,
              "ones_col": np.ones((HD, 1), np.float32),
              "ones_row": np.ones((1, HD), np.float32)}
    if mode == "causal":
        kk = np.arange(128)[:, None]
        qq = np.arange(512)[None, :]
        dmask = np.stack([
            np.where(kk <= qq - 128 * r, 0.0, NEG).astype(np.float32)
            for r in range(4)])
        common["dmask"] = dmask
    elif mode == "full":
        m = np.where(np.isneginf(mask), NEG, mask)
        common["maskT"] = np.ascontiguousarray(m.T)

    in_maps = []
    for c in range(NCORES):
        im = dict(common)
        im["wq"] = np.ascontiguousarray(wq_p[:, c * QD:(c + 1) * QD])
        im["wk"] = np.ascontiguousarray(wk_p[:, c * HD:(c + 1) * HD])
        im["wv"] = np.ascontiguousarray(wv_[:, c * HD:(c + 1) * HD])
        im["wo"] = np.ascontiguousarray(wo_[c * QD:(c + 1) * QD, :])
        in_maps.append(im)
    return mode, in_maps


def _scores_safe(x, wq, wk):
    """The device softmax skips the max-subtraction (scores from
    setup_inputs()-scaled weights are O(5), so exp() is exact and safe).
    Estimate the score magnitude; if exp could overflow fp32, fall back."""
    sx = float(np.sqrt(np.mean(np.square(x), dtype=np.float64)))
    sq = sx * float(np.sqrt(np.mean(np.square(wq), dtype=np.float64)) * np.sqrt(DIM))
    sk = sx * float(np.sqrt(np.mean(np.square(wk), dtype=np.float64)) * np.sqrt(DIM))
    # rope with arbitrary freqs can scale q/k by ~sqrt(2); 7 sigma tail margin
    return 2.0 * sq * sk * 7.0 < 80.0


def _numpy_fallback(x, wq, wk, wv, wo, freqs_cos, freqs_sin, mask):
    """Slow but numerically-safe host path (stable softmax), used only when
    the score magnitudes could overflow the device's unshifted exp."""
    x2 = x.reshape(SEQ, DIM).astype(np.float64)
    q = (x2 @ wq.astype(np.float64)).reshape(SEQ, N_HEADS, HD)
    k = (x2 @ wk.astype(np.float64)).reshape(SEQ, N_KV, HD)
    v = (x2 @ wv.astype(np.float64)).reshape(SEQ, N_KV, HD)
    cos = freqs_cos.astype(np.float64)[:, None, :]
    sin = freqs_sin.astype(np.float64)[:, None, :]

    def rope(t):
        a, b = t[..., 0::2], t[..., 1::2]
        out = np.empty_like(t)
        out[..., 0::2] = a * cos - b * sin
        out[..., 1::2] = a * sin + b * cos
        return out

    q, k = rope(q), rope(k)
    m64 = mask.astype(np.float64)
    outh = np.empty((SEQ, N_HEADS, HD))
    for h in range(N_HEADS):
        g = h // (N_HEADS // N_KV)
        s = q[:, h, :] @ k[:, g, :].T / math.sqrt(HD) + m64
        p = np.exp(s - s.max(-1, keepdims=True))
        p /= p.sum(-1, keepdims=True)
        outh[:, h, :] = p @ v[:, g, :]
    y = outh.reshape(SEQ, N_HEADS * HD) @ wo.astype(np.float64)
    return y.astype(np.float32).reshape(1, SEQ, DIM)


def kernel(x, wq, wk, wv, wo, freqs_cos, freqs_sin, mask, cache_k, cache_v,
           start_pos, **_unused):
    sp = int(np.asarray(start_pos))
    x = np.asarray(x, np.float32)
    wq = np.asarray(wq, np.float32)
    wk = np.asarray(wk, np.float32)
    wv = np.asarray(wv, np.float32)
    wo = np.asarray(wo, np.float32)
    mask = np.asarray(mask, np.float32)
    if sp != 0:
        raise NotImplementedError("kernel assumes start_pos == 0 prefill")
    if not _scores_safe(x, wq, wk):
        return _numpy_fallback(x, wq, wk, wv, wo,
                               np.asarray(freqs_cos, np.float32),
                               np.asarray(freqs_sin, np.float32), mask)

    mode, in_maps = _host_inputs(x, wq, wk, wv, wo,
                                 freqs_cos, freqs_sin, mask)
    nc = get_program(mode)
    res = bass_utils.run_bass_kernel_spmd(nc, in_maps,
                                          core_ids=list(range(NCORES)))
    acc = np.zeros((SEQ, DIM), np.float64)
    for r in res.results:
        acc += r["out"].astype(np.float64)
    return acc.astype(np.float32).reshape(1, SEQ, DIM)



# revision 10
# speedup vs baseline: 1.2603x; 1.2603x over previous
"""Trainium2 Bass kernel: Llama-style attention block (prefill, start_pos=0).

Reference computation (per problem):
  q = x @ wq; k = x @ wk; v = x @ wv          (DIM=4096 -> 32 q-heads / 8 kv-heads, hd=128)
  rope(q, k) with interleaved (even, odd) pairs using freqs_cos/freqs_sin inputs
  scores = q @ k^T / sqrt(128) + mask ; p = softmax(scores) ; o = p @ v (GQA 4x)
  out = o @ wo

Distribution: tensor-parallel over heads on 8 cores. Core c owns q-heads
4c..4c+3 and kv-head c (GQA groups align with the core boundary), i.e.
wq/wk/wv are sharded column-wise and wo row-wise. Each core computes a
full-shape partial of the output projection; the host sums the 8 partials
(the row-parallel all-reduce, done on the host at unshard time).

Performance notes (v2, bf16 dataflow):
  - All matmul operands are bf16: the PE runs 1 cycle/row either way, but
    LDWEIGHTS for a bf16 stationary (107ns) hides completely under the
    previous matmul (213ns), where the fp32 load (224ns) could not. This
    removes the ~60ns/matmul tax the fp32r version paid, and halves DMA.
    Measured end-to-end precision of the full-bf16 pipeline is ~2-3e-3
    scale-relative absmax (tolerance 2e-2); PSUM accumulation stays fp32.
  - softmax-denominator matmuls for a chain pair share one PSUM bank
    (rows 0 and 32); the reciprocal is broadcast across partitions by
    gpsimd.partition_broadcast instead of a PE matmul.
  - attention chains run in head-pairs, software-pipelined (pv/dn for
    block i-1 issue after stp/exp of block i), with the output-projection
    matmuls of the previous q-chunk popped as PE filler inside the chain.
  - output partials are written as fp16 (host sums in fp64), halving the
    32MB/core output write.
"""

import math

import numpy as np
import ml_dtypes

import concourse.bass as bass
import concourse.mybir as mybir
import concourse.tile as tile
from concourse import bacc, bass_utils

DIM = 4096
N_HEADS = 32
N_KV = 8
HD = 128
SEQ = 2048
NCORES = 8
HPC = N_HEADS // NCORES          # q heads per core
QD = HPC * HD                    # 512 q-dims per core
SCALE = 1.0 / math.sqrt(HD)
NEG = -1.0e30

NQC = SEQ // 512                 # q chunks of 512
NKB = SEQ // 128                 # k blocks of 128
NKC = DIM // 128                 # contraction chunks of 128
XG = 2                           # kc chunks per x DMA group

F32 = mybir.dt.float32
F32R = mybir.dt.float32r
BF = mybir.dt.bfloat16
F16 = mybir.dt.float16
EXP = mybir.ActivationFunctionType.Exp
BF_NP = ml_dtypes.bfloat16

_PROG_CACHE = {}
DEBUG_DUMPS = False


def _build_program(mask_mode: str):
    """mask_mode: 'causal' (skip upper blocks, add triangular diagonal
    masks), 'none' (no masking), 'full' (add arbitrary maskT blocks)."""
    assert mask_mode in ("causal", "none", "full")
    nc = bacc.Bacc("TRN2", target_bir_lowering=False, debug=False,
                   num_devices=NCORES)

    xT = nc.dram_tensor("xT", [DIM, SEQ], BF, kind="ExternalInput").ap()
    wq = nc.dram_tensor("wq", [DIM, QD], BF, kind="ExternalInput").ap()
    wk = nc.dram_tensor("wk", [DIM, HD], BF, kind="ExternalInput").ap()
    wv = nc.dram_tensor("wv", [DIM, HD], BF, kind="ExternalInput").ap()
    wo = nc.dram_tensor("wo", [QD, DIM], BF, kind="ExternalInput").ap()
    cos2 = nc.dram_tensor("cos2", [HD, SEQ], BF, kind="ExternalInput").ap()
    sin2 = nc.dram_tensor("sin2", [HD, SEQ], BF, kind="ExternalInput").ap()
    rmat = nc.dram_tensor("rmat", [HD, HD], BF, kind="ExternalInput").ap()
    ident = nc.dram_tensor("ident", [128, 128], BF, kind="ExternalInput").ap()
    ones_col_d = nc.dram_tensor("ones_col", [128, 1], BF,
                                kind="ExternalInput").ap()
    if mask_mode == "causal":
        dmask_d = nc.dram_tensor("dmask", [4, 128, 512], BF,
                                 kind="ExternalInput").ap()
    if mask_mode == "full":
        maskT_d = nc.dram_tensor("maskT", [SEQ, SEQ], F32,
                                 kind="ExternalInput").ap()
    out = nc.dram_tensor("out", [SEQ, DIM], F16, kind="ExternalOutput").ap()
    if DEBUG_DUMPS:
        dbg_qt = nc.dram_tensor("dbg_qt", [HPC, 128, SEQ], BF,
                                kind="ExternalOutput").ap()
        dbg_kt = nc.dram_tensor("dbg_kt", [128, SEQ], BF,
                                kind="ExternalOutput").ap()
        dbg_vs = nc.dram_tensor("dbg_vs", [128, SEQ], BF,
                                kind="ExternalOutput").ap()
        dbg_attn = nc.dram_tensor("dbg_attn", [HPC, 128, SEQ], BF,
                                  kind="ExternalOutput").ap()
        dbg_dn = nc.dram_tensor("dbg_dn", [NQC, 2, 2, 512], F32,
                                kind="ExternalOutput").ap()

    with tile.TileContext(nc) as tc:
        with tc.tile_pool(name="persist", bufs=1) as pp:
            # ---- persistent tiles ----
            qt = [pp.tile([128, SEQ], BF, name=f"qt{h}") for h in range(HPC)]
            kt = pp.tile([128, SEQ], BF)
            vs = pp.tile([128, SEQ], BF)          # seq-major V, block i at cols i*128
            attn = [pp.tile([128, SEQ], BF, name=f"attn{h}")
                    for h in range(HPC)]
            wo_sb = pp.tile([128, HPC, DIM], BF)
            rmat_sb = pp.tile([128, 128], BF)
            ident_sb = pp.tile([128, 128], BF)
            nc.sync.dma_start(ident_sb[:], ident[:])
            nc.sync.dma_start(rmat_sb[:], rmat[:])
            ones_sb = pp.tile([128, 1], BF)
            nc.sync.dma_start(ones_sb[:], ones_col_d[:])
            if mask_mode == "causal":
                dmask_sb = pp.tile([128, 4, 512], BF)
                nc.gpsimd.dma_start(dmask_sb[:],
                                    dmask_d.rearrange("r p q -> p r q"))

            # ================= Phase 1: QKV projections + RoPE =================
            psp = tc.alloc_tile_pool(name="ps", bufs=1, space="PSUM")
            # dummy matmuls on the identity tile keep the PE activity monitor
            # warm through the initial DMA window (else the first ~3.5us of
            # real matmuls run at half clock)
            warm = psp.tile([128, 128], F32, tag="vtr", bufs=1)
            for _ in range(50):
                nc.tensor.matmul(warm[:], ident_sb[:], ident_sb[:],
                                 start=True, stop=True)
            with tc.tile_pool(name="w1", bufs=1) as wp:
                # chunked weight loads so the first matmuls start after ~1MB
                wq_sb = wp.tile([128, NKC, QD], BF)
                wk_sb = wp.tile([128, NKC, HD], BF)
                wv_sb = wp.tile([128, NKC, HD], BF)
                for k0, k1 in ((0, 2), (2, 8), (8, 20), (20, 32)):
                    ksl = slice(k0, k1)
                    for wsb, wdr in ((wq_sb, wq), (wk_sb, wk), (wv_sb, wv)):
                        w3 = wdr.rearrange("(kc p) m -> p kc m", p=128)
                        nc.sync.dma_start(wsb[:, ksl, :], w3[:, ksl, :])
                cos_sb = wp.tile([128, SEQ], BF)
                sin_sb = wp.tile([128, SEQ], BF)

                xT3 = xT.rearrange("(kc p) s -> kc p s", p=128)

                def drain_copy(m, pt, n):
                    # pass A: free the accumulator bank and launch the PE part
                    # (swap matmul / transposes) with nothing else in between,
                    # so neither the DVE nor the PE stream head-of-line blocks
                    # on rope arithmetic of an earlier head
                    raw = wp.tile([128, 512], BF, tag="raw", bufs=6,
                                  name=f"raw{n}_{m}")
                    if m % 2 == 0:
                        nc.scalar.copy(raw[:], pt[:])
                    else:
                        nc.vector.tensor_copy(raw[:], pt[:])
                    if m <= HPC:
                        swp = psp.tile([128, 512], F32,
                                       tag=("aux" if m % 2 == 0 else "vtr"),
                                       bufs=1, name=f"swp{n}_{m}")
                        nc.tensor.matmul(swp[:], rmat_sb[:], raw[:],
                                         start=True, stop=True)
                        return raw, swp
                    pvts = []
                    for b in range(4):
                        pvt = psp.tile([128, 128], BF, tag="vtr", bufs=1,
                                       name=f"pvt{n}_{b}")
                        nc.tensor.transpose(pvt[:], raw[:, b * 128:(b + 1) * 128],
                                            ident_sb[:])
                        pvts.append(pvt)
                    return raw, pvts

                def drain_rope_a(m, raw, pe_out, n, nsl):
                    # pass B1: drain the swap/transpose PSUM banks straight
                    # into the destination (partial rope: dst = swp*sin)
                    if m <= HPC:
                        dst = qt[m] if m < HPC else kt
                        nc.vector.tensor_mul(dst[:, nsl], pe_out[:],
                                             sin_sb[:, nsl])
                        return None
                    for b, pvt in enumerate(pe_out):
                        i = n * 4 + b
                        nc.vector.tensor_copy(vs[:, i * 128:(i + 1) * 128],
                                              pvt[:])
                    return None

                def drain_rope_b(m, raw, tmp, n, nsl):
                    # pass B2: finish the rope combine (dst += raw*cos)
                    if m <= HPC:
                        dst = qt[m] if m < HPC else kt
                        tmp2 = wp.tile([128, 512], BF, tag="ropetmp", bufs=2,
                                       name=f"tmp{n}_{m}")
                        nc.vector.tensor_mul(tmp2[:], raw[:], cos_sb[:, nsl])
                        nc.vector.tensor_add(dst[:, nsl], dst[:, nsl], tmp2[:])

                for n in range(NQC):
                    nsl = slice(n * 512, (n + 1) * 512)
                    mlist = list(range(HPC + 2))
                    pts = {m: psp.tile([128, 512], F32, tag="big", bufs=6,
                                       name=f"pt{n}_{m}")
                           for m in mlist}
                    for g in range(NKC // XG):
                        xg = wp.tile([128, XG, 512], BF, tag="xg", bufs=5)
                        nc.scalar.dma_start(
                            xg[:], xT3[g * XG:(g + 1) * XG, :, nsl]
                            .rearrange("kc p s -> p kc s"))
                        for kk in range(XG):
                            kc = g * XG + kk
                            st, sp = (kc == 0), (kc == NKC - 1)
                            for m in mlist:
                                if m < HPC:
                                    w_ap = wq_sb[:, kc, m * 128:(m + 1) * 128]
                                elif m == HPC:
                                    w_ap = wk_sb[:, kc, :]
                                else:
                                    w_ap = wv_sb[:, kc, :]
                                nc.tensor.matmul(pts[m][:], w_ap,
                                                 xg[:, kk, :],
                                                 start=st, stop=sp)
                    if n == 0:
                        # cos/sin are first needed here; wo/out-proj weights
                        # are needed only in phase 2 — queue both behind the
                        # n=0 x chunks so early HBM bandwidth goes to x/qkv
                        nc.sync.dma_start(cos_sb[:], cos2[:])
                        nc.sync.dma_start(sin_sb[:], sin2[:])
                        wo3 = wo.rearrange("(kc p) n -> p kc n", p=128)
                        for gg in range(2):
                            nc.sync.dma_start(
                                wo_sb[:, :, gg * 2048:(gg + 1) * 2048],
                                wo3[:, :, gg * 2048:(gg + 1) * 2048])
                    handles = {m: drain_copy(m, pts[m], n) for m in mlist}
                    tmps = {m: drain_rope_a(m, *handles[m], n, nsl)
                            for m in mlist}
                    for m in mlist:
                        drain_rope_b(m, handles[m][0], tmps[m], n, nsl)

            # ========== Phase 2: attention chains + out-projection ==========
            # chains run in head pairs, software-pipelined; the previous
            # chunk's out-projection groups are popped as PE filler inside
            # the chains so the PE never waits on the exp chain.
            with tc.tile_pool(name="att", bufs=1) as ap_:

                def wo_groups(j):
                    # out-projection work for seq blocks 4j..4j+3, split into
                    # 32 groups of 4 matmuls + drain copy + output DMA
                    groups = []
                    for m in range(4 * j, 4 * j + 4):
                        msl = slice(m * 128, (m + 1) * 128)
                        for ncol in range(8):
                            def grp(m=m, msl=msl, ncol=ncol):
                                yp = psp.tile([128, 512], F32, tag="big",
                                              bufs=6, name=f"yp{m}_{ncol}")
                                for kc in range(HPC):
                                    nc.tensor.matmul(
                                        yp[:], attn[kc][:, msl],
                                        wo_sb[:, kc, ncol * 512:(ncol + 1) * 512],
                                        start=(kc == 0), stop=(kc == HPC - 1))
                                ysb = ap_.tile([128, 512], F16, tag="ysb",
                                               bufs=6, name=f"ysb{m}_{ncol}")
                                if ncol % 2 == 0:
                                    nc.vector.tensor_copy(ysb[:], yp[:])
                                else:
                                    nc.scalar.copy(ysb[:], yp[:])
                                nc.sync.dma_start(
                                    out[msl, ncol * 512:(ncol + 1) * 512],
                                    ysb[:])
                            groups.append(grp)
                    return groups

                def chain_pair(j, hp, filler, steps_left):
                    jsl = slice(j * 512, (j + 1) * 512)
                    nblk = 4 * j + 4 if mask_mode == "causal" else NKB
                    pv = [psp.tile([128, 512], F32, tag="big", bufs=6,
                                   name=f"pv{hp + c}_{j}") for c in (0, 1)]
                    dnp = psp.tile([128, 512], F32, tag="aux", bufs=1,
                                   name=f"dn{hp}_{j}")
                    prev = None  # (i, off, pexps)

                    def emit_pv_dn(i, off, pexps):
                        st, sp = (i == 0), (i == nblk - 1)
                        for c in (0, 1):
                            nc.tensor.matmul(pv[c][:, off:],
                                             vs[:, i * 128:(i + 1) * 128],
                                             pexps[c][:, off:],
                                             start=st, stop=sp)
                            dnrow = dnp[32 * c:32 * c + 1, off:]
                            nc.tensor.matmul(dnrow, ones_sb[:],
                                             pexps[c][:, off:],
                                             start=st, stop=sp)

                    for i in range(nblk):
                        r = i - 4 * j
                        off = 128 * r if (mask_mode == "causal" and r > 0) else 0
                        qof = j * 512 + off
                        stps = []
                        for c in (0, 1):
                            h = hp + c
                            stp = psp.tile([128, 512], F32, tag="big", bufs=6,
                                           name=f"st{h}_{j}_{i}")
                            nc.tensor.matmul(stp[:, off:],
                                             kt[:, i * 128:(i + 1) * 128],
                                             qt[h][:, qof:(j + 1) * 512],
                                             start=True, stop=True)
                            stps.append(stp)
                        if mask_mode == "causal" and r >= 0:
                            for c in (0, 1):
                                nc.vector.tensor_add(
                                    stps[c][:, off:], stps[c][:, off:],
                                    dmask_sb[:, r, off:])
                        elif mask_mode == "full":
                            mt = ap_.tile([128, 512], F32, tag="mt", bufs=3)
                            nc.sync.dma_start(
                                mt[:], maskT_d[i * 128:(i + 1) * 128, jsl])
                            for c in (0, 1):
                                nc.vector.tensor_add(stps[c][:], stps[c][:],
                                                     mt[:])
                        pexps = []
                        for c in (0, 1):
                            pexp = ap_.tile([128, 512], BF, tag="pexp",
                                            bufs=6, name=f"pexp{hp + c}_{j}_{i}")
                            nc.scalar.activation(pexp[:, off:], stps[c][:, off:],
                                                 EXP, scale=SCALE)
                            pexps.append(pexp)
                        if prev is not None:
                            emit_pv_dn(*prev)
                        # PE filler: out-projection groups of the previous
                        # chunk, spread evenly over the remaining chain steps
                        k = -(-len(filler) // steps_left)  # ceil
                        for _ in range(min(k, len(filler))):
                            filler.pop(0)()
                        steps_left -= 1
                        prev = (i, off, pexps)
                    emit_pv_dn(*prev)

                    for c in (0, 1):
                        dn_src = dnp[0:1, :]
                        if c == 1:
                            # custom-DVE ops misread PSUM at base partition
                            # 32 — stage row 32 through SBUF first
                            dn_src = ap_.tile([1, 512], F32, tag="dns",
                                              bufs=2, name=f"dns{hp}_{j}")
                            nc.vector.tensor_copy(dn_src[:], dnp[32:33, :])
                        rcpr = ap_.tile([1, 512], F32, tag="rcpr", bufs=2,
                                        name=f"rcpr{hp + c}_{j}")
                        nc.vector.reciprocal_approx_fast(rcpr[:], dn_src[:])
                        if DEBUG_DUMPS:
                            dsb = ap_.tile([1, 512], F32, tag="dnd", bufs=2,
                                           name=f"dnd{hp + c}_{j}")
                            nc.vector.tensor_copy(
                                dsb[:], dnp[32 * c:32 * c + 1, :])
                            nc.sync.dma_start(dbg_dn[j, hp // 2, c], dsb[:])
                        bcs = ap_.tile([128, 512], F32, tag="bcs", bufs=2,
                                       name=f"bcs{hp + c}_{j}")
                        nc.gpsimd.partition_broadcast(bcs[:], rcpr[:])
                        nc.vector.tensor_mul(attn[hp + c][:, jsl], pv[c][:],
                                             bcs[:])

                for j in range(NQC):
                    filler = wo_groups(j - 1) if j > 0 else []
                    nblk = 4 * j + 4 if mask_mode == "causal" else NKB
                    for hp in (0, 2):
                        chain_pair(j, hp, filler,
                                   2 * nblk if hp == 0 else nblk)
                    for grp in filler:
                        grp()

                for grp in wo_groups(NQC - 1):
                    grp()
                if DEBUG_DUMPS:
                    for h in range(HPC):
                        nc.sync.dma_start(dbg_qt[h], qt[h][:])
                        nc.sync.dma_start(dbg_attn[h], attn[h][:])
                    nc.sync.dma_start(dbg_kt[:], kt[:])
                    nc.sync.dma_start(dbg_vs[:], vs[:])
            psp.release()

    nc.compile()
    return nc


def get_program(mask_mode: str):
    if mask_mode not in _PROG_CACHE:
        _PROG_CACHE[mask_mode] = _build_program(mask_mode)
    return _PROG_CACHE[mask_mode]


# ====================== host-side preparation ======================

_PERM128 = np.concatenate([np.arange(0, 128, 2), np.arange(1, 128, 2)])


def _perm_cols(w: np.ndarray, n_heads: int) -> np.ndarray:
    """Permute each head's 128 columns: even dims first, odd dims last."""
    cols = np.concatenate([h * 128 + _PERM128 for h in range(n_heads)])
    return w[:, cols]


def _classify_mask(mask: np.ndarray) -> str:
    if not np.any(mask):
        return "none"
    iu = np.triu_indices(SEQ, 1)
    upper = mask[iu]
    lower_ok = not np.any(np.tril(mask))
    upper_ok = bool(np.all(np.isneginf(upper) | (upper <= -1e9)))
    if lower_ok and upper_ok:
        return "causal"
    return "full"


def _bf(a: np.ndarray) -> np.ndarray:
    return np.ascontiguousarray(np.asarray(a, np.float32).astype(BF_NP))


def _host_inputs(x, wq, wk, wv, wo, freqs_cos, freqs_sin, mask):
    x2 = _bf(x.reshape(SEQ, DIM).T)                         # xT [DIM, SEQ]
    wq_p = _bf(_perm_cols(np.asarray(wq, np.float32), N_HEADS))
    wk_p = _bf(_perm_cols(np.asarray(wk, np.float32), N_KV))
    wv_ = _bf(wv)
    wo_ = _bf(wo)

    cosT = np.asarray(freqs_cos, np.float32).T              # [64, SEQ]
    sinT = np.asarray(freqs_sin, np.float32).T
    cos2 = _bf(np.concatenate([cosT, cosT], 0))             # [128, SEQ]
    sin2 = _bf(np.concatenate([sinT, sinT], 0))

    rmat = np.zeros((HD, HD), np.float32)
    rmat[np.arange(64) + 64, np.arange(64)] = -1.0   # swp[:64] = -raw[64:]
    rmat[np.arange(64), np.arange(64) + 64] = 1.0    # swp[64:] = raw[:64]
    ident = np.eye(128, dtype=np.float32)

    mask = np.asarray(mask, np.float32)
    mode = _classify_mask(mask)

    common = {"xT": x2, "cos2": cos2, "sin2": sin2, "rmat": _bf(rmat),
              "ident": _bf(ident),
              "ones_col": _bf(np.ones((HD, 1), np.float32))}
    if mode == "causal":
        kk = np.arange(128)[:, None]
        qq = np.arange(512)[None, :]
        dmask = np.stack([
            np.where(kk <= qq - 128 * r, 0.0, NEG).astype(np.float32)
            for r in range(4)])
        common["dmask"] = _bf(dmask)
    elif mode == "full":
        m = np.where(np.isneginf(mask), NEG, mask)
        common["maskT"] = np.ascontiguousarray(m.T)

    in_maps = []
    for c in range(NCORES):
        im = dict(common)
        im["wq"] = np.ascontiguousarray(wq_p[:, c * QD:(c + 1) * QD])
        im["wk"] = np.ascontiguousarray(wk_p[:, c * HD:(c + 1) * HD])
        im["wv"] = np.ascontiguousarray(wv_[:, c * HD:(c + 1) * HD])
        im["wo"] = np.ascontiguousarray(wo_[c * QD:(c + 1) * QD, :])
        in_maps.append(im)
    return mode, in_maps


def _scores_safe(x, wq, wk):
    """The device softmax skips the max-subtraction (scores from
    setup_inputs()-scaled weights are O(5), so exp() is exact and safe).
    Estimate the score magnitude; if exp could overflow fp32, fall back."""
    sx = float(np.sqrt(np.mean(np.square(x), dtype=np.float64)))
    sq = sx * float(np.sqrt(np.mean(np.square(wq), dtype=np.float64)) * np.sqrt(DIM))
    sk = sx * float(np.sqrt(np.mean(np.square(wk), dtype=np.float64)) * np.sqrt(DIM))
    # rope with arbitrary freqs can scale q/k by ~sqrt(2); 7 sigma tail margin
    return 2.0 * sq * sk * 7.0 < 80.0


def _numpy_fallback(x, wq, wk, wv, wo, freqs_cos, freqs_sin, mask):
    """Slow but numerically-safe host path (stable softmax), used only when
    the score magnitudes could overflow the device's unshifted exp."""
    x2 = x.reshape(SEQ, DIM).astype(np.float64)
    q = (x2 @ wq.astype(np.float64)).reshape(SEQ, N_HEADS, HD)
    k = (x2 @ wk.astype(np.float64)).reshape(SEQ, N_KV, HD)
    v = (x2 @ wv.astype(np.float64)).reshape(SEQ, N_KV, HD)
    cos = freqs_cos.astype(np.float64)[:, None, :]
    sin = freqs_sin.astype(np.float64)[:, None, :]

    def rope(t):
        a, b = t[..., 0::2], t[..., 1::2]
        out = np.empty_like(t)
        out[..., 0::2] = a * cos - b * sin
        out[..., 1::2] = a * sin + b * cos
        return out

    q, k = rope(q), rope(k)
    m64 = mask.astype(np.float64)
    outh = np.empty((SEQ, N_HEADS, HD))
    for h in range(N_HEADS):
        g = h // (N_HEADS // N_KV)
        s = q[:, h, :] @ k[:, g, :].T / math.sqrt(HD) + m64
        p = np.exp(s - s.max(-1, keepdims=True))
        p /= p.sum(-1, keepdims=True)
        outh[:, h, :] = p @ v[:, g, :]
    y = outh.reshape(SEQ, N_HEADS * HD) @ wo.astype(np.float64)
    return y.astype(np.float32).reshape(1, SEQ, DIM)


def kernel(x, wq, wk, wv, wo, freqs_cos, freqs_sin, mask, cache_k, cache_v,
           start_pos, **_unused):
    sp = int(np.asarray(start_pos))
    x = np.asarray(x, np.float32)
    wq = np.asarray(wq, np.float32)
    wk = np.asarray(wk, np.float32)
    wv = np.asarray(wv, np.float32)
    wo = np.asarray(wo, np.float32)
    mask = np.asarray(mask, np.float32)
    if sp != 0:
        raise NotImplementedError("kernel assumes start_pos == 0 prefill")
    if not _scores_safe(x, wq, wk):
        return _numpy_fallback(x, wq, wk, wv, wo,
                               np.asarray(freqs_cos, np.float32),
                               np.asarray(freqs_sin, np.float32), mask)

    mode, in_maps = _host_inputs(x, wq, wk, wv, wo,
                                 freqs_cos, freqs_sin, mask)
    nc = get_program(mode)
    res = bass_utils.run_bass_kernel_spmd(nc, in_maps,
                                          core_ids=list(range(NCORES)))
    acc = np.zeros((SEQ, DIM), np.float64)
    for r in res.results:
        acc += r["out"].astype(np.float64)
    return acc.astype(np.float32).reshape(1, SEQ, DIM)


# revision 15
# speedup vs baseline: 1.3252x; 1.0515x over previous
"""Trainium2 Bass kernel: Llama-style attention block (prefill, start_pos=0).

Reference computation (per problem):
  q = x @ wq; k = x @ wk; v = x @ wv          (DIM=4096 -> 32 q-heads / 8 kv-heads, hd=128)
  rope(q, k) with interleaved (even, odd) pairs using freqs_cos/freqs_sin inputs
  scores = q @ k^T / sqrt(128) + mask ; p = softmax(scores) ; o = p @ v (GQA 4x)
  out = o @ wo

Distribution: tensor-parallel over heads on 8 cores. Core c owns q-heads
4c..4c+3 and kv-head c (GQA groups align with the core boundary), i.e.
wq/wk/wv are sharded column-wise and wo row-wise. Each core computes a
full-shape partial of the output projection; the host sums the 8 partials
(the row-parallel all-reduce, done on the host at unshard time).

Performance notes (v2, bf16 dataflow):
  - All matmul operands are bf16: the PE runs 1 cycle/row either way, but
    LDWEIGHTS for a bf16 stationary (107ns) hides completely under the
    previous matmul (213ns), where the fp32 load (224ns) could not. This
    removes the ~60ns/matmul tax the fp32r version paid, and halves DMA.
    Measured end-to-end precision of the full-bf16 pipeline is ~2-3e-3
    scale-relative absmax (tolerance 2e-2); PSUM accumulation stays fp32.
  - softmax-denominator matmuls for a chain pair share one PSUM bank
    (rows 0 and 32); the reciprocal is broadcast across partitions by
    gpsimd.partition_broadcast instead of a PE matmul.
  - attention chains run in head-pairs, software-pipelined (pv/dn for
    block i-1 issue after stp/exp of block i), with the output-projection
    matmuls of the previous q-chunk popped as PE filler inside the chain.
  - output partials are written as fp16 (host sums in fp64), halving the
    32MB/core output write.
"""

import math

import numpy as np
import ml_dtypes

import concourse.bass as bass
import concourse.mybir as mybir
import concourse.tile as tile
from concourse import bacc, bass_utils

DIM = 4096
N_HEADS = 32
N_KV = 8
HD = 128
SEQ = 2048
NCORES = 8
HPC = N_HEADS // NCORES          # q heads per core
QD = HPC * HD                    # 512 q-dims per core
SCALE = 1.0 / math.sqrt(HD)
NEG = -1.0e30

NQC = SEQ // 512                 # q chunks of 512
NKB = SEQ // 128                 # k blocks of 128
NKC = DIM // 128                 # contraction chunks of 128
XG = 2                           # kc chunks per x DMA group

F32 = mybir.dt.float32
F32R = mybir.dt.float32r
BF = mybir.dt.bfloat16
F16 = mybir.dt.float16
EXP = mybir.ActivationFunctionType.Exp
BF_NP = ml_dtypes.bfloat16

_PROG_CACHE = {}
DEBUG_DUMPS = False


def _build_program(mask_mode: str):
    """mask_mode: 'causal' (skip upper blocks, add triangular diagonal
    masks), 'none' (no masking), 'full' (add arbitrary maskT blocks)."""
    assert mask_mode in ("causal", "none", "full")
    nc = bacc.Bacc("TRN2", target_bir_lowering=False, debug=False,
                   num_devices=NCORES)

    xT = nc.dram_tensor("xT", [DIM, SEQ], BF, kind="ExternalInput").ap()
    wq = nc.dram_tensor("wq", [DIM, QD], BF, kind="ExternalInput").ap()
    wk = nc.dram_tensor("wk", [DIM, HD], BF, kind="ExternalInput").ap()
    wv = nc.dram_tensor("wv", [DIM, HD], BF, kind="ExternalInput").ap()
    wo = nc.dram_tensor("wo", [QD, DIM], BF, kind="ExternalInput").ap()
    cos2 = nc.dram_tensor("cos2", [HD, SEQ], BF, kind="ExternalInput").ap()
    sin2 = nc.dram_tensor("sin2", [HD, SEQ], BF, kind="ExternalInput").ap()
    rmat = nc.dram_tensor("rmat", [HD, HD], BF, kind="ExternalInput").ap()
    ident = nc.dram_tensor("ident", [128, 128], BF, kind="ExternalInput").ap()
    ones_col_d = nc.dram_tensor("ones_col", [128, 1], BF,
                                kind="ExternalInput").ap()
    ones_r_d = nc.dram_tensor("ones_r", [128, 1], F32R,
                              kind="ExternalInput").ap()
    if mask_mode == "causal":
        dmask_d = nc.dram_tensor("dmask", [4, 128, 512], BF,
                                 kind="ExternalInput").ap()
    if mask_mode == "full":
        maskT_d = nc.dram_tensor("maskT", [SEQ, SEQ], F32,
                                 kind="ExternalInput").ap()
    out = nc.dram_tensor("out", [SEQ, DIM], F16, kind="ExternalOutput").ap()
    if DEBUG_DUMPS:
        dbg_qt = nc.dram_tensor("dbg_qt", [HPC, 128, SEQ], BF,
                                kind="ExternalOutput").ap()
        dbg_kt = nc.dram_tensor("dbg_kt", [128, SEQ], BF,
                                kind="ExternalOutput").ap()
        dbg_vs = nc.dram_tensor("dbg_vs", [128, SEQ], BF,
                                kind="ExternalOutput").ap()
        dbg_attn = nc.dram_tensor("dbg_attn", [HPC, 128, SEQ], BF,
                                  kind="ExternalOutput").ap()
        dbg_dn = nc.dram_tensor("dbg_dn", [NQC, 2, 2, 512], F32,
                                kind="ExternalOutput").ap()

    with tile.TileContext(nc) as tc:
        with tc.tile_pool(name="persist", bufs=1) as pp:
            # ---- persistent tiles ----
            qt = [pp.tile([128, SEQ], BF, name=f"qt{h}") for h in range(HPC)]
            kt = pp.tile([128, SEQ], BF)
            vs = pp.tile([128, SEQ], BF)          # seq-major V, block i at cols i*128
            # dim-major V chunks for n>=1; their seq-major transposes are
            # deferred into the j=0 attention window as PE filler
            rawv = [pp.tile([128, 512], BF, name=f"rawv{n}")
                    for n in range(1, NQC)]
            attn = [pp.tile([128, SEQ], BF, name=f"attn{h}")
                    for h in range(HPC)]
            wo_sb = pp.tile([128, HPC, DIM], BF)
            rmat_sb = pp.tile([128, 128], BF)
            ident_sb = pp.tile([128, 128], BF)
            nc.sync.dma_start(ident_sb[:], ident[:])
            nc.sync.dma_start(rmat_sb[:], rmat[:])
            ones_sb = pp.tile([128, 1], BF)
            nc.sync.dma_start(ones_sb[:], ones_col_d[:])
            ones_sr = pp.tile([128, 1], F32R)
            nc.sync.dma_start(ones_sr[:], ones_r_d[:])
            if mask_mode == "causal":
                dmask_sb = pp.tile([128, 4, 512], BF)
                nc.gpsimd.dma_start(dmask_sb[:],
                                    dmask_d.rearrange("r p q -> p r q"))

            # ================= Phase 1: QKV projections + RoPE =================
            psp = tc.alloc_tile_pool(name="ps", bufs=1, space="PSUM")
            # dummy matmuls on the identity tile keep the PE activity monitor
            # warm through the initial DMA window (else the first ~3.5us of
            # real matmuls run at half clock)
            warm = psp.tile([128, 128], F32, tag="vtr", bufs=1)
            for _ in range(50):
                nc.tensor.matmul(warm[:], ident_sb[:], ident_sb[:],
                                 start=True, stop=True)
            with tc.tile_pool(name="w1", bufs=1) as wp:
                # chunked weight loads so the first matmuls start after ~1MB
                wq_sb = wp.tile([128, NKC, QD], BF)
                wk_sb = wp.tile([128, NKC, HD], BF)
                wv_sb = wp.tile([128, NKC, HD], BF)
                for k0, k1 in ((0, 1), (1, 4), (4, 12), (12, 32)):
                    ksl = slice(k0, k1)
                    for wsb, wdr in ((wq_sb, wq), (wk_sb, wk), (wv_sb, wv)):
                        w3 = wdr.rearrange("(kc p) m -> p kc m", p=128)
                        nc.sync.dma_start(wsb[:, ksl, :], w3[:, ksl, :])
                cos_sb = wp.tile([128, SEQ], BF)
                sin_sb = wp.tile([128, SEQ], BF)

                xT3 = xT.rearrange("(kc p) s -> kc p s", p=128)

                def drain_copy(m, pt, n):
                    # pass A: free the accumulator bank and launch the PE part
                    # (swap matmul / transposes) with nothing else in between,
                    # so neither the DVE nor the PE stream head-of-line blocks
                    # on rope arithmetic of an earlier head
                    if m == HPC + 1 and n > 0:
                        # V chunk for n>=1: drain dim-major copy only; the
                        # seq-major transpose runs as j=0 attention filler
                        nc.scalar.copy(rawv[n - 1][:], pt[:])
                        return rawv[n - 1], []
                    raw = wp.tile([128, 512], BF, tag="raw", bufs=6,
                                  name=f"raw{n}_{m}")
                    if m % 2 == 0:
                        nc.scalar.copy(raw[:], pt[:])
                    else:
                        nc.vector.tensor_copy(raw[:], pt[:])
                    if m <= HPC:
                        swp = psp.tile([128, 512], F32,
                                       tag=("aux" if m % 2 == 0 else "vtr"),
                                       bufs=1, name=f"swp{n}_{m}")
                        nc.tensor.matmul(swp[:], rmat_sb[:], raw[:],
                                         start=True, stop=True)
                        return raw, swp
                    pvts = []
                    for b in range(4):
                        pvt = psp.tile([128, 128], BF, tag="vtr", bufs=1,
                                       name=f"pvt{n}_{b}")
                        nc.tensor.transpose(pvt[:], raw[:, b * 128:(b + 1) * 128],
                                            ident_sb[:])
                        pvts.append(pvt)
                    return raw, pvts

                def drain_rope_a(m, raw, pe_out, n, nsl):
                    # pass B1: drain the swap/transpose PSUM banks straight
                    # into the destination (partial rope: dst = swp*sin)
                    if m <= HPC:
                        dst = qt[m] if m < HPC else kt
                        nc.vector.tensor_mul(dst[:, nsl], pe_out[:],
                                             sin_sb[:, nsl])
                        return None
                    for b, pvt in enumerate(pe_out):
                        i = n * 4 + b
                        nc.vector.tensor_copy(vs[:, i * 128:(i + 1) * 128],
                                              pvt[:])
                    return None

                def drain_rope_b(m, raw, tmp, n, nsl):
                    # pass B2: finish the rope combine (dst += raw*cos)
                    if m <= HPC:
                        dst = qt[m] if m < HPC else kt
                        tmp2 = wp.tile([128, 512], BF, tag="ropetmp", bufs=2,
                                       name=f"tmp{n}_{m}")
                        nc.vector.tensor_mul(tmp2[:], raw[:], cos_sb[:, nsl])
                        nc.vector.tensor_add(dst[:, nsl], dst[:, nsl], tmp2[:])

                for n in range(NQC):
                    nsl = slice(n * 512, (n + 1) * 512)
                    mlist = list(range(HPC + 2))
                    pts = {m: psp.tile([128, 512], F32, tag="big", bufs=6,
                                       name=f"pt{n}_{m}")
                           for m in mlist}
                    for g in range(NKC // XG):
                        xg = wp.tile([128, XG, 512], BF, tag="xg", bufs=5)
                        nc.scalar.dma_start(
                            xg[:], xT3[g * XG:(g + 1) * XG, :, nsl]
                            .rearrange("kc p s -> p kc s"))
                        for kk in range(XG):
                            kc = g * XG + kk
                            st, sp = (kc == 0), (kc == NKC - 1)
                            for m in mlist:
                                if m < HPC:
                                    w_ap = wq_sb[:, kc, m * 128:(m + 1) * 128]
                                elif m == HPC:
                                    w_ap = wk_sb[:, kc, :]
                                else:
                                    w_ap = wv_sb[:, kc, :]
                                nc.tensor.matmul(pts[m][:], w_ap,
                                                 xg[:, kk, :],
                                                 start=st, stop=sp)
                    if n == 0:
                        # cos/sin are first needed here
                        nc.sync.dma_start(cos_sb[:], cos2[:])
                        nc.sync.dma_start(sin_sb[:], sin2[:])
                    if n == 1:
                        # out-proj weights are needed only in phase 2
                        wo3 = wo.rearrange("(kc p) n -> p kc n", p=128)
                        for gg in range(2):
                            nc.sync.dma_start(
                                wo_sb[:, :, gg * 2048:(gg + 1) * 2048],
                                wo3[:, :, gg * 2048:(gg + 1) * 2048])
                    handles = {m: drain_copy(m, pts[m], n) for m in mlist}
                    tmps = {m: drain_rope_a(m, *handles[m], n, nsl)
                            for m in mlist}
                    for m in mlist:
                        drain_rope_b(m, handles[m][0], tmps[m], n, nsl)

            # ========== Phase 2: attention chains + out-projection ==========
            # chains run in head pairs, software-pipelined; the previous
            # chunk's out-projection groups are popped as PE filler inside
            # the chains so the PE never waits on the exp chain.
            with tc.tile_pool(name="att", bufs=1) as ap_:

                def vtr_groups():
                    # deferred seq-major V transposes for chunks 1..3 —
                    # PE filler for the j=0 chains
                    groups = []
                    for n in range(1, NQC):
                        for b in range(4):
                            def grp(n=n, b=b):
                                i = n * 4 + b
                                pvt = psp.tile([128, 128], BF, tag="vtr",
                                               bufs=1, name=f"dpvt{n}_{b}")
                                nc.tensor.transpose(
                                    pvt[:], rawv[n - 1][:, b * 128:(b + 1) * 128],
                                    ident_sb[:])
                                if b % 2 == 0:
                                    nc.vector.tensor_copy(
                                        vs[:, i * 128:(i + 1) * 128], pvt[:])
                                else:
                                    nc.scalar.copy(
                                        vs[:, i * 128:(i + 1) * 128], pvt[:])
                            groups.append(grp)
                    return groups

                def wo_groups(j):
                    # out-projection work for seq blocks 4j..4j+3, split into
                    # 32 groups of 4 matmuls + drain copy + output DMA
                    groups = []
                    for m in range(4 * j, 4 * j + 4):
                        msl = slice(m * 128, (m + 1) * 128)
                        for ncol in range(8):
                            def grp(m=m, msl=msl, ncol=ncol):
                                yp = psp.tile([128, 512], F32, tag="big",
                                              bufs=6, name=f"yp{m}_{ncol}")
                                for kc in range(HPC):
                                    nc.tensor.matmul(
                                        yp[:], attn[kc][:, msl],
                                        wo_sb[:, kc, ncol * 512:(ncol + 1) * 512],
                                        start=(kc == 0), stop=(kc == HPC - 1))
                                ysb = ap_.tile([128, 512], F16, tag="ysb",
                                               bufs=6, name=f"ysb{m}_{ncol}")
                                if ncol % 2 == 0:
                                    nc.vector.tensor_copy(ysb[:], yp[:])
                                else:
                                    nc.scalar.copy(ysb[:], yp[:])
                                nc.sync.dma_start(
                                    out[msl, ncol * 512:(ncol + 1) * 512],
                                    ysb[:])
                            groups.append(grp)
                    return groups

                def chain_pair(j, hp, filler, steps_left):
                    jsl = slice(j * 512, (j + 1) * 512)
                    nblk = 4 * j + 4 if mask_mode == "causal" else NKB
                    dve_dn = j > 0   # j=0 has no PE filler; keep dn on PE there
                    pv = [psp.tile([128, 512], F32, tag="big", bufs=6,
                                   name=f"pv{hp + c}_{j}") for c in (0, 1)]
                    dnp = psp.tile([128, 512], F32, tag="aux", bufs=1,
                                   name=f"dn{hp}_{j}")
                    acc = [ap_.tile([128, 512], F32R, tag="acc", bufs=4,
                                    name=f"acc{hp + c}_{j}") for c in (0, 1)]
                    prev = None  # (i, off, pexps)

                    def emit_pv_dn(i, off, pexps):
                        st, sp = (i == 0), (i == nblk - 1)
                        for c in (0, 1):
                            nc.tensor.matmul(pv[c][:, off:],
                                             vs[:, i * 128:(i + 1) * 128],
                                             pexps[c][:, off:],
                                             start=st, stop=sp)
                            if not dve_dn:
                                dnrow = dnp[32 * c:32 * c + 1, off:]
                                nc.tensor.matmul(dnrow, ones_sb[:],
                                                 pexps[c][:, off:],
                                                 start=st, stop=sp)
                            elif st:
                                nc.vector.tensor_copy(acc[c][:], pexps[c][:])
                            else:
                                nc.vector.tensor_add(acc[c][:, off:],
                                                     acc[c][:, off:],
                                                     pexps[c][:, off:])

                    for i in range(nblk):
                        r = i - 4 * j
                        off = 128 * r if (mask_mode == "causal" and r > 0) else 0
                        qof = j * 512 + off
                        stps = []
                        for c in (0, 1):
                            h = hp + c
                            stp = psp.tile([128, 512], F32, tag="big", bufs=6,
                                           name=f"st{h}_{j}_{i}")
                            nc.tensor.matmul(stp[:, off:],
                                             kt[:, i * 128:(i + 1) * 128],
                                             qt[h][:, qof:(j + 1) * 512],
                                             start=True, stop=True)
                            stps.append(stp)
                        if mask_mode == "causal" and r >= 0:
                            for c in (0, 1):
                                nc.vector.tensor_add(
                                    stps[c][:, off:], stps[c][:, off:],
                                    dmask_sb[:, r, off:])
                        elif mask_mode == "full":
                            mt = ap_.tile([128, 512], F32, tag="mt", bufs=3)
                            nc.sync.dma_start(
                                mt[:], maskT_d[i * 128:(i + 1) * 128, jsl])
                            for c in (0, 1):
                                nc.vector.tensor_add(stps[c][:], stps[c][:],
                                                     mt[:])
                        pexps = []
                        for c in (0, 1):
                            pexp = ap_.tile([128, 512], BF, tag="pexp",
                                            bufs=6, name=f"pexp{hp + c}_{j}_{i}")
                            nc.scalar.activation(pexp[:, off:], stps[c][:, off:],
                                                 EXP, scale=SCALE)
                            pexps.append(pexp)
                        if prev is not None:
                            emit_pv_dn(*prev)
                        # PE filler: out-projection groups of the previous
                        # chunk, spread evenly over the remaining chain steps
                        k = -(-len(filler) // steps_left)  # ceil
                        for _ in range(min(k, len(filler))):
                            filler.pop(0)()
                        steps_left -= 1
                        prev = (i, off, pexps)
                    emit_pv_dn(*prev)

                    if dve_dn:
                        for c in (0, 1):
                            # fp32 matmuls fail the ISA check at output base
                            # partition 32; one bf16 rounding of acc before a
                            # 128-way fp32 sum costs ~0.02% on dn
                            accb = ap_.tile([128, 512], BF, tag="accb",
                                            bufs=2, name=f"accb{hp + c}_{j}")
                            nc.vector.tensor_copy(accb[:], acc[c][:])
                            nc.tensor.matmul(dnp[32 * c:32 * c + 1, :],
                                             ones_sb[:], accb[:],
                                             start=True, stop=True)
                    for c in (0, 1):
                        dn_src = dnp[0:1, :]
                        if c == 1:
                            # custom-DVE ops misread PSUM at base partition
                            # 32 — stage row 32 through SBUF first
                            dn_src = ap_.tile([1, 512], F32, tag="dns",
                                              bufs=2, name=f"dns{hp}_{j}")
                            nc.vector.tensor_copy(dn_src[:], dnp[32:33, :])
                        rcpr = ap_.tile([1, 512], F32, tag="rcpr", bufs=2,
                                        name=f"rcpr{hp + c}_{j}")
                        nc.vector.reciprocal_approx_fast(rcpr[:], dn_src[:])
                        if DEBUG_DUMPS:
                            dsb = ap_.tile([1, 512], F32, tag="dnd", bufs=2,
                                           name=f"dnd{hp + c}_{j}")
                            nc.vector.tensor_copy(
                                dsb[:], dnp[32 * c:32 * c + 1, :])
                            nc.sync.dma_start(dbg_dn[j, hp // 2, c], dsb[:])
                        bcs = ap_.tile([128, 512], F32, tag="bcs", bufs=2,
                                       name=f"bcs{hp + c}_{j}")
                        nc.gpsimd.partition_broadcast(bcs[:], rcpr[:])
                        nc.vector.tensor_mul(attn[hp + c][:, jsl], pv[c][:],
                                             bcs[:])

                for j in range(NQC):
                    filler = wo_groups(j - 1) if j > 0 else vtr_groups()
                    nblk = 4 * j + 4 if mask_mode == "causal" else NKB
                    for hp in (0, 2):
                        chain_pair(j, hp, filler,
                                   2 * nblk if hp == 0 else nblk)
                    for grp in filler:
                        grp()

                for grp in wo_groups(NQC - 1):
                    grp()
                if DEBUG_DUMPS:
                    for h in range(HPC):
                        nc.sync.dma_start(dbg_qt[h], qt[h][:])
                        nc.sync.dma_start(dbg_attn[h], attn[h][:])
                    nc.sync.dma_start(dbg_kt[:], kt[:])
                    nc.sync.dma_start(dbg_vs[:], vs[:])
            psp.release()

    nc.compile()
    return nc


def get_program(mask_mode: str):
    if mask_mode not in _PROG_CACHE:
        _PROG_CACHE[mask_mode] = _build_program(mask_mode)
    return _PROG_CACHE[mask_mode]


# ====================== host-side preparation ======================

_PERM128 = np.concatenate([np.arange(0, 128, 2), np.arange(1, 128, 2)])


def _perm_cols(w: np.ndarray, n_heads: int) -> np.ndarray:
    """Permute each head's 128 columns: even dims first, odd dims last."""
    cols = np.concatenate([h * 128 + _PERM128 for h in range(n_heads)])
    return w[:, cols]


def _classify_mask(mask: np.ndarray) -> str:
    if not np.any(mask):
        return "none"
    iu = np.triu_indices(SEQ, 1)
    upper = mask[iu]
    lower_ok = not np.any(np.tril(mask))
    upper_ok = bool(np.all(np.isneginf(upper) | (upper <= -1e9)))
    if lower_ok and upper_ok:
        return "causal"
    return "full"


def _bf(a: np.ndarray) -> np.ndarray:
    return np.ascontiguousarray(np.asarray(a, np.float32).astype(BF_NP))


def _host_inputs(x, wq, wk, wv, wo, freqs_cos, freqs_sin, mask):
    x2 = _bf(x.reshape(SEQ, DIM).T)                         # xT [DIM, SEQ]
    wq_p = _bf(_perm_cols(np.asarray(wq, np.float32), N_HEADS))
    wk_p = _bf(_perm_cols(np.asarray(wk, np.float32), N_KV))
    wv_ = _bf(wv)
    wo_ = _bf(wo)

    cosT = np.asarray(freqs_cos, np.float32).T              # [64, SEQ]
    sinT = np.asarray(freqs_sin, np.float32).T
    cos2 = _bf(np.concatenate([cosT, cosT], 0))             # [128, SEQ]
    sin2 = _bf(np.concatenate([sinT, sinT], 0))

    rmat = np.zeros((HD, HD), np.float32)
    rmat[np.arange(64) + 64, np.arange(64)] = -1.0   # swp[:64] = -raw[64:]
    rmat[np.arange(64), np.arange(64) + 64] = 1.0    # swp[64:] = raw[:64]
    ident = np.eye(128, dtype=np.float32)

    mask = np.asarray(mask, np.float32)
    mode = _classify_mask(mask)

    common = {"xT": x2, "cos2": cos2, "sin2": sin2, "rmat": _bf(rmat),
              "ident": _bf(ident),
              "ones_col": _bf(np.ones((HD, 1), np.float32)),
              "ones_r": np.ones((HD, 1), np.float32)}
    if mode == "causal":
        kk = np.arange(128)[:, None]
        qq = np.arange(512)[None, :]
        dmask = np.stack([
            np.where(kk <= qq - 128 * r, 0.0, NEG).astype(np.float32)
            for r in range(4)])
        common["dmask"] = _bf(dmask)
    elif mode == "full":
        m = np.where(np.isneginf(mask), NEG, mask)
        common["maskT"] = np.ascontiguousarray(m.T)

    in_maps = []
    for c in range(NCORES):
        im = dict(common)
        im["wq"] = np.ascontiguousarray(wq_p[:, c * QD:(c + 1) * QD])
        im["wk"] = np.ascontiguousarray(wk_p[:, c * HD:(c + 1) * HD])
        im["wv"] = np.ascontiguousarray(wv_[:, c * HD:(c + 1) * HD])
        im["wo"] = np.ascontiguousarray(wo_[c * QD:(c + 1) * QD, :])
        in_maps.append(im)
    return mode, in_maps


def _scores_safe(x, wq, wk):
    """The device softmax skips the max-subtraction (scores from
    setup_inputs()-scaled weights are O(5), so exp() is exact and safe).
    Estimate the score magnitude; if exp could overflow fp32, fall back."""
    sx = float(np.sqrt(np.mean(np.square(x), dtype=np.float64)))
    sq = sx * float(np.sqrt(np.mean(np.square(wq), dtype=np.float64)) * np.sqrt(DIM))
    sk = sx * float(np.sqrt(np.mean(np.square(wk), dtype=np.float64)) * np.sqrt(DIM))
    # rope with arbitrary freqs can scale q/k by ~sqrt(2); 7 sigma tail margin
    return 2.0 * sq * sk * 7.0 < 80.0


def _numpy_fallback(x, wq, wk, wv, wo, freqs_cos, freqs_sin, mask):
    """Slow but numerically-safe host path (stable softmax), used only when
    the score magnitudes could overflow the device's unshifted exp."""
    x2 = x.reshape(SEQ, DIM).astype(np.float64)
    q = (x2 @ wq.astype(np.float64)).reshape(SEQ, N_HEADS, HD)
    k = (x2 @ wk.astype(np.float64)).reshape(SEQ, N_KV, HD)
    v = (x2 @ wv.astype(np.float64)).reshape(SEQ, N_KV, HD)
    cos = freqs_cos.astype(np.float64)[:, None, :]
    sin = freqs_sin.astype(np.float64)[:, None, :]

    def rope(t):
        a, b = t[..., 0::2], t[..., 1::2]
        out = np.empty_like(t)
        out[..., 0::2] = a * cos - b * sin
        out[..., 1::2] = a * sin + b * cos
        return out

    q, k = rope(q), rope(k)
    m64 = mask.astype(np.float64)
    outh = np.empty((SEQ, N_HEADS, HD))
    for h in range(N_HEADS):
        g = h // (N_HEADS // N_KV)
        s = q[:, h, :] @ k[:, g, :].T / math.sqrt(HD) + m64
        p = np.exp(s - s.max(-1, keepdims=True))
        p /= p.sum(-1, keepdims=True)
        outh[:, h, :] = p @ v[:, g, :]
    y = outh.reshape(SEQ, N_HEADS * HD) @ wo.astype(np.float64)
    return y.astype(np.float32).reshape(1, SEQ, DIM)


def kernel(x, wq, wk, wv, wo, freqs_cos, freqs_sin, mask, cache_k, cache_v,
           start_pos, **_unused):
    sp = int(np.asarray(start_pos))
    x = np.asarray(x, np.float32)
    wq = np.asarray(wq, np.float32)
    wk = np.asarray(wk, np.float32)
    wv = np.asarray(wv, np.float32)
    wo = np.asarray(wo, np.float32)
    mask = np.asarray(mask, np.float32)
    if sp != 0:
        raise NotImplementedError("kernel assumes start_pos == 0 prefill")
    if not _scores_safe(x, wq, wk):
        return _numpy_fallback(x, wq, wk, wv, wo,
                               np.asarray(freqs_cos, np.float32),
                               np.asarray(freqs_sin, np.float32), mask)

    mode, in_maps = _host_inputs(x, wq, wk, wv, wo,
                                 freqs_cos, freqs_sin, mask)
    nc = get_program(mode)
    res = bass_utils.run_bass_kernel_spmd(nc, in_maps,
                                          core_ids=list(range(NCORES)))
    acc = np.zeros((SEQ, DIM), np.float64)
    for r in res.results:
        acc += r["out"].astype(np.float64)
    return acc.astype(np.float32).reshape(1, SEQ, DIM)
